# revision 3
# baseline (speedup 1.0000x reference)
"""Trainium2 Bass kernel for nn_ExtractPatchesPositionLayer.

Reference semantics: per image b, bilinear-translate the (522,522,1) padded
object by t = -positions[b] (tfa.translate: out(y,x) = img(y+py, x+px),
zero fill outside), then center-crop 5px -> (512,512,1).

The shift is constant per image, so floor/frac of the offset give an integer
window start (A,B) plus four bilinear corner weights. The host extracts each
image's integer-aligned 513x513 window (zero-padded at the borders, row-padded
to 514 for even alignment) and casts it to fp16 — after that every device
access pattern is STATIC, so all DMAs are plain HWDGE copies that spray across
all 16 SDMA engines (the previous dynamic-offset DMAs all serialized on one
engine/queue, which was the 1.4 ms bottleneck).

Blocked layout: each SBUF partition p holds 5 consecutive window rows
(4 output rows + 1 halo) contiguous in DRAM -> ~5 KB load packets, and both
bilinear blends become free-dim shifts:

    v[p, k*514 + c] = (1-wy) * w[p, k*514 + c] + wy * w[p, (k+1)*514 + c]
    h[p, k*514 + j] = (1-wx) * v[p, k*514 + j] + wx * v[p, k*514 + j + 1]

Each blend is one tensor_scalar_mul (DVE 2-4x mode) + one fused
scalar_tensor_tensor (DVE 2x mode) over the whole [128, ~2056] tile; a single
ACT copy casts/compacts fp16 -> fp32 [128, 4*512], and the store writes 8 KB
contiguous per partition. No matmul, no PSUM, no communication.
Sharding: batch 256 -> 32 images x 8 cores, embarrassingly parallel.
"""

from dataclasses import dataclass

import numpy as np

import concourse.bacc as bacc
import concourse.bass as bass
import concourse.mybir as mybir
import concourse.tile as tile
from concourse.bass_utils import run_bass_kernel_spmd

PAD = 5


@dataclass(frozen=True)
class Cfg:
    bpc: int   # images per core
    n: int     # output height/width (512)

    @property
    def win(self):  # window rows/cols actually used
        return self.n + 1

    @property
    def rs(self):   # row stride in the staged window (win padded to even)
        return self.win + 1

    @property
    def rpp(self):  # output rows per partition
        return self.n // 128


def build_nc(cfg: Cfg) -> bass.Bass:
    BPC, N, RS = cfg.bpc, cfg.n, cfg.rs
    K = cfg.rpp                 # 4 output rows per partition
    IMG = cfg.win * RS          # elems per staged image (513*514)
    NN = N * N                  # elems per output image
    FB = K * RS                 # blend width (4*514 = 2056)
    FH = FB - 2                 # horizontal width (2054, even)
    f16 = mybir.dt.float16
    f32 = mybir.dt.float32
    mult = mybir.AluOpType.mult
    add = mybir.AluOpType.add

    nc = bacc.Bacc("TRN2", target_bir_lowering=False, debug=False)
    x_d = nc.declare_dram_parameter("x", [BPC, IMG], f16, isOutput=False)
    wm_d = nc.declare_dram_parameter("wm", [128, BPC * 4], f32, isOutput=False)
    y_d = nc.declare_dram_parameter("y", [BPC, NN], f32, isOutput=True)

    with tile.TileContext(nc) as tc:
        with (
            tc.tile_pool(name="const", bufs=1) as constp,
            tc.tile_pool(name="win", bufs=3) as winp,
            tc.tile_pool(name="mid", bufs=2) as midp,
            tc.tile_pool(name="outp", bufs=3) as outp,
        ):
            wm_sb = constp.tile([128, BPC * 4], f32, tag="wm")
            nc.sync.dma_start(wm_sb[:], wm_d[:, :])

            for b in range(BPC):
                # partition p <- window rows K*p .. K*p+K (halo row shared
                # with partition p+1); contiguous 5*RS-elem read per partition
                w = winp.tile([128, (K + 1) * RS], f16, tag="w")
                nc.sync.dma_start(
                    w[:], bass.AP(x_d, b * IMG, [[K * RS, 128], [1, (K + 1) * RS]])
                )
                onemy = wm_sb[:, 4 * b + 0: 4 * b + 1]
                wyv = wm_sb[:, 4 * b + 1: 4 * b + 2]
                onemx = wm_sb[:, 4 * b + 2: 4 * b + 3]
                wxv = wm_sb[:, 4 * b + 3: 4 * b + 4]

                # The fused scalar_tensor_tensor op only has a 1x DVE uop, so
                # each blend is decomposed into two single-src muls (2-4x DVE
                # mode / other engines) + one f16 tensor_tensor add (2x mode),
                # spread so no engine exceeds the DMA roofline:
                #   DVE: two muls + both adds, ACT: one mul + cast, GPSIMD:
                #   one mul (1-input ops run near line-rate there).
                # vertical blend: v = (1-wy)*w_k + wy*w_{k+1}
                m0 = midp.tile([128, FB], f16, tag="m0")
                m1 = midp.tile([128, FB], f16, tag="m1")
                v = midp.tile([128, FB], f16, tag="v")
                nc.vector.tensor_scalar_mul(m0[:], w[:, 0:FB], onemy)
                nc.gpsimd.tensor_scalar_mul(m1[:], w[:, RS:RS + FB], wyv)
                nc.vector.tensor_tensor(v[:], m0[:], m1[:], add)

                # horizontal blend: h = (1-wx)*v_j + wx*v_{j+1}
                # (the +1-shifted read rides the single-src DVE mul, whose
                #  2x_2p mode has no alignment requirement)
                m2 = midp.tile([128, FH], f16, tag="m2")
                m3 = midp.tile([128, FH], f16, tag="m3")
                h = midp.tile([128, FB], f16, tag="h")
                nc.scalar.mul(m2[:], v[:, 0:FH], onemx)
                nc.vector.tensor_scalar_mul(m3[:], v[:, 1:1 + FH], wxv)
                nc.vector.tensor_tensor(h[:, 0:FH], m2[:], m3[:], add)

                # cast fp16 -> fp32 and compact away the RS padding columns
                o = outp.tile([128, K * N], f32, tag="o")
                h3 = h[:].rearrange("p (k w) -> p k w", w=RS)
                nc.scalar.copy(
                    o[:].rearrange("p (k w) -> p k w", w=N), h3[:, :, 0:N])
                # partition p -> output rows K*p .. K*p+K-1 (8 KB contiguous)
                nc.scalar.dma_start(
                    bass.AP(y_d, b * NN, [[K * N, 128], [1, K * N]]), o[:])
    nc.compile()
    return nc


def host_prep(padded: np.ndarray, positions: np.ndarray, n_cores: int):
    """Shard + stage integer-aligned fp16 windows.

    padded: (B, npad, npad) f32, positions: (B, 2)."""
    B, npad, _ = padded.shape
    n = npad - 2 * PAD
    cfg = Cfg(bpc=B // n_cores, n=n)
    win, rs = cfg.win, cfg.rs

    px = positions[:, 0].astype(np.float64)
    py = positions[:, 1].astype(np.float64)
    fy = np.floor(py)
    fx = np.floor(px)
    ay = (PAD + fy).astype(np.int64)
    ax = (PAD + fx).astype(np.int64)
    wy = (py - fy).astype(np.float32)
    wx = (px - fx).astype(np.float32)

    xw = np.zeros((B, win, rs), dtype=np.float16)
    for b in range(B):
        r0 = max(int(ay[b]), 0)
        r1 = min(int(ay[b]) + win, npad)
        c0 = max(int(ax[b]), 0)
        c1 = min(int(ax[b]) + win, npad)
        if r1 > r0 and c1 > c0:
            xw[b, r0 - ay[b]:r1 - ay[b], c0 - ax[b]:c1 - ax[b]] = \
                padded[b, r0:r1, c0:c1]

    bpc = cfg.bpc
    in_maps = []
    for cidx in range(n_cores):
        sl = slice(cidx * bpc, (cidx + 1) * bpc)
        wmat = np.empty((128, bpc * 4), dtype=np.float32)
        wmat[:, 0::4] = (1.0 - wy[sl])[None, :]
        wmat[:, 1::4] = wy[sl][None, :]
        wmat[:, 2::4] = (1.0 - wx[sl])[None, :]
        wmat[:, 3::4] = wx[sl][None, :]
        in_maps.append({
            "x": xw[sl].reshape(bpc, win * rs),
            "wm": wmat,
        })
    return cfg, in_maps


N_CORES = 8
_nc_cache: dict = {}


def kernel(padded_obj: np.ndarray, positions: np.ndarray) -> np.ndarray:
    padded_obj = np.asarray(padded_obj)
    positions = np.asarray(positions)
    B, npad, _, C = padded_obj.shape
    cfg, in_maps = host_prep(
        padded_obj.reshape(B, npad, npad).astype(np.float32, copy=False),
        positions, N_CORES)

    nc = _nc_cache.get(cfg)
    if nc is None:
        nc = build_nc(cfg)
        _nc_cache[cfg] = nc

    res = run_bass_kernel_spmd(nc, in_maps, core_ids=list(range(N_CORES)))
    out = np.concatenate([r["y"] for r in res.results], axis=0)
    return out.reshape(B, cfg.n, cfg.n, 1).astype(np.float32, copy=False)


# revision 5
# speedup vs baseline: 6.8074x; 6.8074x over previous
"""Trainium2 Bass kernel for nn_ExtractPatchesPositionLayer.

Reference semantics: per image b, bilinear-translate the (522,522,1) padded
object by t = -positions[b] (tfa.translate: out(y,x) = img(y+py, x+px),
zero fill outside), then center-crop 5px -> (512,512,1).

The shift is constant per image, so floor/frac of the offset give an integer
window start (A,B) plus four bilinear corner weights c00,c01,c10,c11. The
host extracts each image's integer-aligned 513x513 window (zero-padded at the
borders, row-padded to 514 for even alignment) and casts it to fp16 — after
that every device access pattern is STATIC, so all DMAs are plain HWDGE
copies that spray evenly across all 16 SDMA engines (dynamic-offset DMAs all
serialize on one engine/queue, which was the original 1.4 ms bottleneck).

Blocked layout: SBUF partition p holds 5 consecutive window rows (4 output
rows + 1 halo row) contiguous in DRAM -> ~5 KB load packets, and BOTH
bilinear taps become free-dim shifts of the same tile:

    out[p, k, j] = c00*w[p, k*RS+j]   + c01*w[p, k*RS+j+1]
                 + c10*w[p,(k+1)*RS+j] + c11*w[p,(k+1)*RS+j+1]

which the (otherwise idle) tensor engine evaluates as 4 accumulating
matmuls per 512-wide chunk with SCALED-IDENTITY stationary weights
(lhsT = c_ij * I): out = sum_ij (c_ij I)^T @ shifted_view(w). PSUM
accumulates in fp32, so the fp16->fp32 output cast is free; DVE/ACT only
build the tiny scaled identities and copy PSUM->SBUF. The store writes 8 KB
contiguous per partition. Sharding: batch 256 -> 32 images x 8 cores,
embarrassingly parallel, no communication.
"""

from dataclasses import dataclass

import numpy as np

import concourse.bacc as bacc
import concourse.bass as bass
import concourse.mybir as mybir
import concourse.tile as tile
from concourse.bass_utils import run_bass_kernel_spmd

PAD = 5


@dataclass(frozen=True)
class Cfg:
    bpc: int   # images per core
    n: int     # output height/width (512)

    @property
    def win(self):  # window rows/cols actually used
        return self.n + 1

    @property
    def rs(self):   # row stride in the staged window (win padded to even)
        return self.win + 1

    @property
    def rpp(self):  # output rows per partition
        return self.n // 128


def build_nc(cfg: Cfg) -> bass.Bass:
    BPC, N, RS = cfg.bpc, cfg.n, cfg.rs
    K = cfg.rpp                 # 4 output rows per partition
    IMG = cfg.win * RS          # elems per staged image (513*514)
    NN = N * N                  # elems per output image
    NK = N                      # matmul chunk width (512 output cols per k)
    f16 = mybir.dt.float16
    f32 = mybir.dt.float32

    nc = bacc.Bacc("TRN2", target_bir_lowering=False, debug=False)
    x_d = nc.declare_dram_parameter("x", [BPC, IMG], f16, isOutput=False)
    wm_d = nc.declare_dram_parameter("wm", [128, BPC * 4], f32, isOutput=False)
    id_d = nc.declare_dram_parameter("idm", [128, 128], f16, isOutput=False)
    y_d = nc.declare_dram_parameter("y", [BPC, NN], f32, isOutput=True)

    with tile.TileContext(nc) as tc:
        with (
            tc.tile_pool(name="const", bufs=1) as constp,
            tc.tile_pool(name="win", bufs=3) as winp,
            tc.tile_pool(name="lt", bufs=2) as ltp,
            tc.tile_pool(name="outp", bufs=3) as outp,
            tc.tile_pool(name="ps", bufs=2, space="PSUM") as psp,
        ):
            wm_sb = constp.tile([128, BPC * 4], f32, tag="wm")
            nc.sync.dma_start(wm_sb[:], wm_d[:, :])
            id_sb = constp.tile([128, 128], f16, tag="idm")
            nc.sync.dma_start(id_sb[:], id_d[:, :])

            for b in range(BPC):
                # partition p <- window rows K*p .. K*p+K (halo row shared
                # with partition p+1); contiguous 5*RS-elem read per partition
                w = winp.tile([128, (K + 1) * RS], f16, tag="w")
                nc.sync.dma_start(
                    w[:], bass.AP(x_d, b * IMG, [[K * RS, 128], [1, (K + 1) * RS]])
                )

                # stationary weights: lhsT_ij = c_ij * I  (tiny DVE muls)
                lts = []
                for ij in range(4):
                    lt = ltp.tile([128, 128], f16, tag=f"lt{ij}")
                    nc.vector.tensor_scalar_mul(
                        lt[:], id_sb[:], wm_sb[:, 4 * b + ij: 4 * b + ij + 1])
                    lts.append(lt)

                # 4 shifted taps x 4 chunks; grouped by lhsT so the PE keeps
                # each weight matrix loaded for 4 consecutive matmuls
                ps = psp.tile([128, K * NK], f32, tag="ps")
                shift = [0, 1, RS, RS + 1]
                for ij in range(4):
                    for k in range(K):
                        nc.tensor.matmul(
                            out=ps[:, k * NK:(k + 1) * NK],
                            lhsT=lts[ij][:],
                            rhs=w[:, k * RS + shift[ij]: k * RS + shift[ij] + NK],
                            start=(ij == 0), stop=(ij == 3))

                # PSUM -> SBUF (fp32), split across DVE and ACT
                o = outp.tile([128, K * NK], f32, tag="o")
                half = K * NK // 2
                nc.vector.tensor_copy(o[:, 0:half], ps[:, 0:half])
                nc.scalar.copy(o[:, half:], ps[:, half:])
                # partition p -> output rows K*p .. K*p+K-1 (8 KB contiguous)
                nc.scalar.dma_start(
                    bass.AP(y_d, b * NN, [[K * NK, 128], [1, K * NK]]), o[:])
    nc.compile()
    return nc


def host_prep(padded: np.ndarray, positions: np.ndarray, n_cores: int):
    """Shard + stage integer-aligned fp16 windows.

    padded: (B, npad, npad) f32, positions: (B, 2)."""
    B, npad, _ = padded.shape
    n = npad - 2 * PAD
    cfg = Cfg(bpc=B // n_cores, n=n)
    win, rs = cfg.win, cfg.rs

    px = positions[:, 0].astype(np.float64)
    py = positions[:, 1].astype(np.float64)
    fy = np.floor(py)
    fx = np.floor(px)
    ay = (PAD + fy).astype(np.int64)
    ax = (PAD + fx).astype(np.int64)
    wy = (py - fy).astype(np.float32)
    wx = (px - fx).astype(np.float32)

    xw = np.zeros((B, win, rs), dtype=np.float16)
    for b in range(B):
        r0 = max(int(ay[b]), 0)
        r1 = min(int(ay[b]) + win, npad)
        c0 = max(int(ax[b]), 0)
        c1 = min(int(ax[b]) + win, npad)
        if r1 > r0 and c1 > c0:
            xw[b, r0 - ay[b]:r1 - ay[b], c0 - ax[b]:c1 - ax[b]] = \
                padded[b, r0:r1, c0:c1]

    bpc = cfg.bpc
    idm = np.eye(128, dtype=np.float16)
    in_maps = []
    for cidx in range(n_cores):
        sl = slice(cidx * bpc, (cidx + 1) * bpc)
        wmat = np.empty((128, bpc * 4), dtype=np.float32)
        wmat[:, 0::4] = ((1 - wy[sl]) * (1 - wx[sl]))[None, :]  # c00: no shift
        wmat[:, 1::4] = ((1 - wy[sl]) * wx[sl])[None, :]        # c01: +1 col
        wmat[:, 2::4] = (wy[sl] * (1 - wx[sl]))[None, :]        # c10: +1 row
        wmat[:, 3::4] = (wy[sl] * wx[sl])[None, :]              # c11: both
        in_maps.append({
            "x": xw[sl].reshape(bpc, win * rs),
            "wm": wmat,
            "idm": idm,
        })
    return cfg, in_maps


N_CORES = 8
_nc_cache: dict = {}


def kernel(padded_obj: np.ndarray, positions: np.ndarray) -> np.ndarray:
    padded_obj = np.asarray(padded_obj)
    positions = np.asarray(positions)
    B, npad, _, C = padded_obj.shape
    cfg, in_maps = host_prep(
        padded_obj.reshape(B, npad, npad).astype(np.float32, copy=False),
        positions, N_CORES)

    nc = _nc_cache.get(cfg)
    if nc is None:
        nc = build_nc(cfg)
        _nc_cache[cfg] = nc

    res = run_bass_kernel_spmd(nc, in_maps, core_ids=list(range(N_CORES)))
    out = np.concatenate([r["y"] for r in res.results], axis=0)
    return out.reshape(B, cfg.n, cfg.n, 1).astype(np.float32, copy=False)
